# revision 19
# baseline (speedup 1.0000x reference)
"""Trainium2 Bass kernel for nn_ConvGraph_SC (gnn_message_passing).

Reference computation (per batch b of 64, N=32 nodes, C=512 channels, 7x7 spatial):
    state = input.mean(axis=(3,4))                       # [B, N, C]
    mat1  = state @ W1.T + b1
    mat2  = state @ W2.T + b2
    adj   = mat1 @ mat2.T                                # [B, N, N]
    soft  = softmax((adj - mean(adj)) / std(adj), rows)  # global mean/std, ddof=1
    out   = mean(soft @ state + state, axis=1)           # [B, C]

Device-side algebra (S = spatial SUM of x, unscaled):
  * softmax((adj-mu)/sigma) is invariant to a per-batch affine rescale of adj,
    so we work with adj' = 2401*adj = S A S^T + 49*s_u 1^T + 49*1 s_v^T + c0'
    where A = W1^T W2, u' = 49 W1^T b2, v' = 49 W2^T b1, c0' = 2401 b1.b2.
  * Row-constant terms (s_u, c0) drop out of the row softmax; they enter only
    the global mean/std, computed from per-row sums with closed-form
    corrections. No row-max subtraction before exp: after global mean/std
    normalization the argument is bounded (|z| ~< 10), safe in f32.
  * x is shipped fp16 (rel-err budget 2e-2; lands ~5e-4) -> HBM traffic
    halves to ~12.9 MB/core + 0.55 MB weights (~36.5us stream at ~360GB/s).
  * Per-partition row layout is [s(49), c_low(128)]: the spatial-sum
    pairwise-add tree runs 3 contiguous 4B-aligned in-place fp16
    tensor_tensor ops per batch at DVE 2x mode, folding 49 slabs to 6
    partials + the untouched s24 slab.  The PE consumes those 7 pieces
    directly: the transpose (st) and the output matmul are both linear in
    S, so the summed sraw never materializes and the expensive small tree
    levels never run.
  * TA = A^T S^T is batched per stats-group on the PE (16 wide matmuls +
    one PSUM->SBUF copy per group instead of per batch).
  * adj row sums / sums-of-squares / q run on the Scalar engine
    (activation accum_out) to keep DVE on trees.
  * Epilogue per group: soft = exp-scaled rows (Scalar, per-row 1/rowsum),
    vcol = soft^T 1 (one PE matmul per batch), wbc[p] = vcol[p//4] via a
    host-built gather-mask matmul, wf4 = (mask4/(N*HW)) * wbc (Scalar), and
    out accumulates wf4-weighted pieces + the residual mask4/(N*HW)-weighted
    pieces on the PE.  One output copy + one DMA per group.
  * Stats groups [4, 3, 1]: only batch 7's chain trails the last input
    byte.

Sharding: pure data parallel, 8 batches per NeuronCore, weights replicated.
"""

import numpy as np

import concourse.bacc as bacc
import concourse.tile as tile
from concourse import masks, mybir
from concourse.bass_utils import run_bass_kernel_spmd

F32 = mybir.dt.float32
F16 = mybir.dt.float16
I32 = mybir.dt.int32
NCORES = 8
B, N, C, HW = 64, 32, 512, 49
BPC = B // NCORES          # batches per core
FREE = N * C * HW // 128   # 6272 fp16 elems per partition per batch
SLAB = 128                 # one spatial position = 128 c_low elems
K1023 = float(np.sqrt(np.float64(1023.0)))
GSIZE = [4, 3, 1]
GBASE = [0, 4, 7]
GRP_OF_B = [0, 0, 0, 0, 1, 1, 1, 2]
BP_OF_B = [0, 1, 2, 3, 0, 1, 2, 0]
# weights blob columns (fp16): amat 0:2048, uv 2048:2056, c0 2056,
# G32 gather mask (rows 0:32) 2058:2186, mask4/(N*HW) 2186:2190
WCOLS = 2192

_CACHED_NC = None

A_ = mybir.AluOpType


def build_bass():
    nc = bacc.Bacc("TRN2", target_bir_lowering=False)

    x_d = nc.declare_dram_parameter("x", [BPC, 128, FREE], F16, isOutput=False)
    w_d = nc.declare_dram_parameter("wblob", [128, WCOLS], F16, isOutput=False)
    out_d = nc.declare_dram_parameter("out", [4, 128 * BPC], F32, isOutput=True)

    lp = nc.allow_low_precision("fp16 input + spatial sums; rel-err budget 2e-2")
    lp.__enter__()
    with tile.TileContext(nc) as tc:
        with (
            tc.tile_pool(name="singles", bufs=1) as singles,
            tc.tile_pool(name="tap", bufs=2) as tap,
            tc.tile_pool(name="small", bufs=2) as small,
            tc.tile_pool(name="ps_t", bufs=1, space="PSUM") as ps_t_pool,
            tc.tile_pool(name="ps_tt", bufs=2, space="PSUM") as ps_tt_pool,
            tc.tile_pool(name="ps_adj", bufs=2, space="PSUM") as ps_adj_pool,
            tc.tile_pool(name="ps_misc", bufs=2, space="PSUM") as ps_misc_pool,
            tc.tile_pool(name="ps_out", bufs=1, space="PSUM") as ps_out_pool,
        ):
            # ---- persistent tiles -----------------------------------------
            x_all = singles.tile([128, FREE * BPC], F16)
            ident = singles.tile([128, 128], F16)
            ones16 = singles.tile([1, 128], F16)
            ones_c16 = singles.tile([32, 1], F16)
            ones32 = singles.tile([32, 32], F32)
            wsb = singles.tile([128, WCOLS], F16)
            c0_sb = singles.tile([32, 1], F32)
            # state^T: [p=c_low, 128b + 4n + c_hi], fp16
            st_all = singles.tile([128, 128 * BPC], F16)
            outsb = singles.tile([4, 128 * BPC], F32)

            a_sb = wsb[:, 0:2048]
            uv_sb = wsb[:, 2048:2056]
            g32 = wsb[0:32, 2058:2186]   # gather mask G[k, p] = (k == p//4)
            m4s = wsb[:, 2186:2190]      # mask4 / (N*HW)

            def load_weights():
                # one blob DMA on the scalar HWDGE ring so the sync ring
                # carries only the x stream
                nc.scalar.dma_start(out=wsb[:], in_=w_d[:])
                nc.scalar.copy(c0_sb[:], wsb[0:32, 2056:2057])
                masks.make_identity(nc, ident[:])
                nc.gpsimd.memset(ones16[:], 1.0)
                nc.gpsimd.memset(ones_c16[:], 1.0)
                nc.gpsimd.memset(ones32[:], 1.0)

            # per-group state (allocated lazily in program order)
            grp = {}

            def start_group(g):
                gs = GSIZE[g]
                grp[g] = {
                    # adj [:32, 0:128]; sv rows [0:1, 128+32bp : 160+32bp]
                    "ps_adj": ps_adj_pool.tile([32, 256], F32, name="ps_adj"),
                    # ps_misc regions: su cols [:32, 0:4], stats bcast
                    # [:32, 16:24], vcol [:32, 32+bp], wbc [:, 64+bp]
                    "ps_misc": ps_misc_pool.tile([128, 128], F32, name="ps_misc"),
                    "ps_out": ps_out_pool.tile([4, 128 * gs], F32, name="ps_out"),
                    "ps_tt": ps_tt_pool.tile([128, 128 * gs], F32, name="ps_tt"),
                    "ta": tap.tile([128, 128 * gs], F16, tag="ta", name="ta"),
                    "sv": small.tile([1, 32 * gs], F16, tag="sv", name="sv"),
                    "q": small.tile([32, gs], F32, tag="q", name="q"),
                    "t": small.tile([32, gs], F32, tag="t", name="t"),
                    "rq": small.tile([32, gs], F32, tag="rq", name="rq"),
                    "expt": small.tile([32, 32 * gs], F32, tag="expt", name="expt"),
                }

            def dma_batch(b):
                nc.sync.dma_start(
                    out=x_all[:, FREE * b : FREE * (b + 1)], in_=x_d[b, :, :]
                )

            def tree(b):
                # fold 49 slabs to 6 partial slabs fully in place (2x-mode
                # contiguous fp16 adds); s24 stays untouched; the PE consumes
                # the 7 pieces directly
                xb = x_all[:, FREE * b : FREE * (b + 1)]
                v = nc.vector
                v.tensor_add(xb[:, 0:3072], xb[:, 0:3072], xb[:, 3200:6272])
                v.tensor_add(xb[:, 0:1536], xb[:, 0:1536], xb[:, 1536:3072])
                v.tensor_add(xb[:, 0:768], xb[:, 0:768], xb[:, 768:1536])

            def pieces(b):
                xb = x_all[:, FREE * b : FREE * (b + 1)]
                return [xb[:, 128 * k : 128 * (k + 1)] for k in range(6)] + [
                    xb[:, 3072:3200]
                ]

            def st_gslice(g, r):
                gb, gs = GBASE[g], GSIZE[g]
                return st_all[:, 128 * gb + r : 128 * (gb + gs) : 4]

            def chain(b):
                # per-batch PE work: piece transposes -> st, su/sv
                g, bp = GRP_OF_B[b], BP_OF_B[b]
                gd = grp[g]
                scol = slice(128 * b, 128 * (b + 1))

                ps_t = ps_t_pool.tile([128, 128], F32)
                pcs = pieces(b)
                for k, pc in enumerate(pcs):
                    nc.tensor.matmul(
                        ps_t[:], pc, ident[:],
                        start=(k == 0), stop=(k == len(pcs) - 1),
                    )
                nc.scalar.copy(st_all[:, scol], ps_t[:])

                st_b = [
                    st_all[:, 128 * b + r : 128 * (b + 1) : 4] for r in range(4)
                ]
                ps_misc = gd["ps_misc"]
                for r in range(4):
                    nc.tensor.matmul(
                        ps_misc[:32, bp : bp + 1],
                        st_b[r],
                        uv_sb[:, 2 * r : 2 * r + 1],
                        start=(r == 0), stop=(r == 3),
                    )
                svsl = slice(128 + 32 * bp, 128 + 32 * (bp + 1))
                for r in range(4):
                    nc.tensor.matmul(
                        gd["ps_adj"][0:1, svsl],
                        uv_sb[:, 2 * r + 1 : 2 * r + 2],
                        st_b[r],
                        start=(r == 0), stop=(r == 3),
                    )
                nc.scalar.copy(
                    gd["sv"][:, 32 * bp : 32 * (bp + 1)], gd["ps_adj"][0:1, svsl]
                )

            def group_adj(g):
                # TA^T = A^T S^T for the whole group in 16 wide matmuls,
                # one PSUM->SBUF copy, then per-batch adj + scalar stats
                gd = grp[g]
                gs = GSIZE[g]
                ps_tt, ta_g = gd["ps_tt"], gd["ta"]
                W = 32 * gs
                for s in range(4):
                    for r in range(4):
                        nc.tensor.matmul(
                            ps_tt[:, W * s : W * (s + 1)],
                            a_sb[:, 512 * r + 128 * s : 512 * r + 128 * (s + 1)],
                            st_gslice(g, r),
                            start=(r == 0), stop=(r == 3),
                        )
                nc.scalar.copy(ta_g[:], ps_tt[:])

                ps_adj, ps_misc = gd["ps_adj"], gd["ps_misc"]
                for bp in range(gs):
                    b = GBASE[g] + bp
                    st_b = [
                        st_all[:, 128 * b + r : 128 * (b + 1) : 4]
                        for r in range(4)
                    ]
                    asl = slice(32 * bp, 32 * (bp + 1))
                    for s in range(4):
                        nc.tensor.matmul(
                            ps_adj[:, asl],
                            ta_g[:, W * s + 32 * bp : W * s + 32 * (bp + 1)],
                            st_b[s],
                            start=(s == 0), stop=False,
                        )
                    nc.tensor.matmul(
                        ps_adj[:, asl],
                        ones16[0:1, 0:32],
                        gd["sv"][0:1, 32 * bp : 32 * (bp + 1)],
                        start=False, stop=True,
                    )
                    # per-batch stats on the Scalar engine
                    id_scr = small.tile([32, 32], F32, tag="id_scr", name="i_s")
                    nc.scalar.activation(
                        out=id_scr[:], in_=ps_adj[:, asl],
                        func=mybir.ActivationFunctionType.Identity,
                        accum_out=gd["t"][:, bp : bp + 1],
                    )
                    sq_scr = small.tile([32, 32], F32, tag="sq_scr", name="s_s")
                    nc.scalar.activation(
                        out=sq_scr[:], in_=ps_adj[:, asl],
                        func=mybir.ActivationFunctionType.Square,
                        accum_out=gd["rq"][:, bp : bp + 1],
                    )
                    nc.scalar.activation(
                        out=gd["q"][:, bp : bp + 1], in_=ps_misc[:32, bp : bp + 1],
                        func=mybir.ActivationFunctionType.Identity,
                        bias=c0_sb[:], scale=1.0,
                    )

            def finish_stats_a(g):
                # S1/S2 of the true adj' from row sums; one colsum-broadcast
                # matmul (all-ones stationary) + one copy back to SBUF
                gd = grp[g]
                gs = GSIZE[g]
                ps_misc = gd["ps_misc"]
                q_g, t_g, rowsq = gd["q"], gd["t"], gd["rq"]

                stats_g = small.tile([32, 2 * gs], F32, tag="stats_g", name="sg")
                nc.vector.scalar_tensor_tensor(
                    out=stats_g[:, 0:gs], in0=q_g[:], scalar=32.0, in1=t_g[:],
                    op0=A_.mult, op1=A_.add,
                )
                h_g = small.tile([32, gs], F32, tag="h_g", name="h_g")
                nc.vector.tensor_add(h_g[:], t_g[:], stats_g[:, 0:gs])
                s2c = small.tile([32, gs], F32, tag="s2c", name="s2c")
                nc.vector.tensor_mul(s2c[:], q_g[:], h_g[:])
                nc.vector.tensor_add(stats_g[:, gs : 2 * gs], rowsq[:], s2c[:])
                nc.tensor.matmul(
                    ps_misc[:32, 16 : 16 + 2 * gs], ones32[:], stats_g[:],
                    start=True, stop=True,
                )
                s_all = small.tile([32, 2 * gs], F32, tag="s_all", name="s_all")
                nc.scalar.copy(s_all[:], ps_misc[:32, 16 : 16 + 2 * gs])
                gd["s_all"] = s_all

            def finish_stats_b(g):
                # inv_std = sqrt(1023)/sqrt(S2 - S1^2/1024), Newton rsqrt
                # with magic seed, 1 iteration (~2e-3 worst-case rel err;
                # budget is 2e-2); exp stays the only activation table
                gd = grp[g]
                gs = GSIZE[g]
                s_all = gd["s_all"]
                t1 = small.tile([32, gs], F32, tag="t1", name="t1")
                nc.vector.tensor_mul(t1[:], s_all[:, 0:gs], s_all[:, 0:gs])
                nc.vector.tensor_scalar(
                    out=t1[:], in0=t1[:], scalar1=-1.0 / 1024.0, scalar2=None,
                    op0=A_.mult,
                )
                v1023 = small.tile([32, gs], F32, tag="v1023", name="v1023")
                nc.vector.tensor_add(v1023[:], t1[:], s_all[:, gs : 2 * gs])
                yint = small.tile([32, gs], I32, tag="yint", name="yint")
                nc.vector.tensor_scalar(
                    out=yint[:], in0=v1023[:].bitcast(I32), scalar1=1,
                    scalar2=None, op0=A_.logical_shift_right,
                )
                nc.vector.tensor_scalar(
                    out=yint[:], in0=yint[:], scalar1=-1,
                    scalar2=0x5F3759DF, op0=A_.mult, op1=A_.add,
                )
                y = small.tile([32, gs], F32, tag="y", name="y")
                nc.vector.tensor_copy(y[:], yint[:].bitcast(F32))
                ya = small.tile([32, gs], F32, tag="ya", name="ya")
                yb = small.tile([32, gs], F32, tag="yb", name="yb")
                nc.vector.tensor_mul(ya[:], y[:], y[:])
                nc.vector.tensor_mul(yb[:], ya[:], v1023[:])
                nc.vector.tensor_scalar(
                    out=ya[:], in0=yb[:], scalar1=-0.5 * K1023,
                    scalar2=1.5 * K1023, op0=A_.mult, op1=A_.add,
                )
                nc.vector.tensor_mul(y[:], y[:], ya[:])
                gd["inv"] = y

            def finish_epi(g):
                # softmax + epilogue: soft rows on Scalar, vcol = soft^T 1 on
                # PE, wbc = gather(vcol) via the G32 mask matmul, wf4 on
                # Scalar, out accumulates wf4- and m4s-weighted pieces
                gd = grp[g]
                gs = GSIZE[g]
                ps_adj, ps_misc, ps_out = gd["ps_adj"], gd["ps_misc"], gd["ps_out"]
                inv_g = gd["inv"]
                expt = gd["expt"]
                for bp in range(gs):
                    nc.scalar.activation(
                        out=expt[:, 32 * bp : 32 * (bp + 1)],
                        in_=ps_adj[:, 32 * bp : 32 * (bp + 1)],
                        func=mybir.ActivationFunctionType.Exp,
                        bias=0.0, scale=inv_g[:, bp : bp + 1],
                    )
                rowsum = small.tile([32, gs], F32, tag="rowsum", name="rs")
                nc.vector.reduce_sum(
                    out=rowsum[:],
                    in_=expt[:, 0 : 32 * gs].rearrange("p (b m) -> p b m", m=32),
                    axis=mybir.AxisListType.X,
                )
                recip = small.tile([32, gs], F32, tag="recip", name="recip")
                nc.vector.reciprocal(recip[:], rowsum[:])
                soft = small.tile([32, 32 * gs], F16, tag="soft", name="soft")
                for bp in range(gs):
                    nc.scalar.activation(
                        out=soft[:, 32 * bp : 32 * (bp + 1)],
                        in_=expt[:, 32 * bp : 32 * (bp + 1)],
                        func=mybir.ActivationFunctionType.Identity,
                        scale=recip[:, bp : bp + 1],
                    )
                # vcol[k, bp] = colsum(soft_bp)[k]
                for bp in range(gs):
                    nc.tensor.matmul(
                        ps_misc[:32, 32 + bp : 33 + bp],
                        soft[:, 32 * bp : 32 * (bp + 1)],
                        ones_c16[:],
                        start=True, stop=True,
                    )
                vcol = small.tile([32, gs], F16, tag="vcol", name="vcol")
                nc.scalar.copy(vcol[:], ps_misc[:32, 32 : 32 + gs])
                # wbc[p, bp] = vcol[p//4, bp] on all 128 partitions
                nc.tensor.matmul(
                    ps_misc[:, 64 : 64 + gs], g32[:], vcol[:],
                    start=True, stop=True,
                )
                wbc = small.tile([128, gs], F32, tag="wbc", name="wbc")
                nc.scalar.copy(wbc[:], ps_misc[:, 64 : 64 + gs])
                wf4a = small.tile([128, 4 * gs], F16, tag="wf4", name="wf4")
                for bp in range(gs):
                    nc.scalar.activation(
                        out=wf4a[:, 4 * bp : 4 * (bp + 1)], in_=m4s[:],
                        func=mybir.ActivationFunctionType.Identity,
                        scale=wbc[:, bp : bp + 1],
                    )
                for bp in range(gs):
                    b = GBASE[g] + bp
                    pcs = pieces(b)
                    npc = len(pcs)
                    for k, pc in enumerate(pcs):
                        nc.tensor.matmul(
                            ps_out[:4, 128 * bp : 128 * (bp + 1)],
                            wf4a[:, 4 * bp : 4 * (bp + 1)],
                            pc,
                            start=(k == 0), stop=False,
                        )
                    for k, pc in enumerate(pcs):
                        nc.tensor.matmul(
                            ps_out[:4, 128 * bp : 128 * (bp + 1)],
                            m4s[:],
                            pc,
                            start=False, stop=(k == npc - 1),
                        )
                gb = GBASE[g]
                nc.scalar.copy(
                    outsb[:, 128 * gb : 128 * (gb + gs)], ps_out[:4, 0 : 128 * gs]
                )
                nc.sync.dma_start(
                    out=out_d[:, 128 * gb : 128 * (gb + gs)],
                    in_=outsb[:, 128 * gb : 128 * (gb + gs)],
                )

            # ---- schedule --------------------------------------------------
            load_weights()
            for b in range(BPC):
                dma_batch(b)

            start_group(0)
            for b in (0, 1, 2, 3):
                tree(b)
                chain(b)
            group_adj(0)
            start_group(1)
            tree(4)
            chain(4)
            tree(5)
            chain(5)
            finish_stats_a(0)
            tree(6)
            chain(6)
            group_adj(1)
            start_group(2)
            finish_stats_b(0)
            tree(7)
            finish_epi(0)
            chain(7)
            finish_stats_a(1)
            finish_stats_b(1)
            finish_epi(1)
            group_adj(2)
            finish_stats_a(2)
            finish_stats_b(2)
            finish_epi(2)

    lp.__exit__(None, None, None)
    nc.finalize()
    return nc


def host_prep(input, W1, b1, W2, b2):
    # x: [B, N, C, 7, 7] f32 -> fp16, partition p = 4n + c_hi, per-partition
    # row layout [s(49), c_low(128)]
    x = np.asarray(input, dtype=np.float32)
    xt = (
        x.reshape(B, N, 4, 128, HW)
        .transpose(0, 1, 2, 4, 3)
        .astype(np.float16)
        .reshape(B, 128, FREE)
    )
    w1 = np.asarray(W1, dtype=np.float64)
    w2 = np.asarray(W2, dtype=np.float64)
    b1 = np.asarray(b1, dtype=np.float64)
    b2 = np.asarray(b2, dtype=np.float64)
    # softmax((adj-mu)/sigma) is scale-invariant per batch: use 2401*adj so A
    # stays in fp16-normal range
    amat = (w1.T @ w2).astype(np.float16)
    u = HW * (w1.T @ b2)
    v = HW * (w2.T @ b1)
    uv = np.stack([u, v], axis=1).astype(np.float16)
    c0 = float(HW * HW * (b1 @ b2))
    p = np.arange(128)
    g32m = (np.arange(32)[:, None] == (p[None, :] // 4)).astype(np.float16)
    m4s = ((np.arange(4)[None, :] == (p[:, None] % 4)) / (N * HW)).astype(
        np.float16
    )

    wblob = np.zeros((128, WCOLS), dtype=np.float16)
    wblob[:, 0:2048] = amat.reshape(4, 128, 512).transpose(1, 0, 2).reshape(128, 2048)
    wblob[:, 2048:2056] = uv.reshape(4, 128, 2).transpose(1, 0, 2).reshape(128, 8)
    wblob[:, 2056] = c0
    wblob[0:32, 2058:2186] = g32m
    wblob[:, 2186:2190] = m4s
    return xt, np.ascontiguousarray(wblob)


def make_in_maps(input, W1, b1, W2, b2):
    xt, wblob = host_prep(input, W1, b1, W2, b2)
    in_maps = []
    for i in range(NCORES):
        shard = np.ascontiguousarray(xt[BPC * i : BPC * (i + 1)])
        in_maps.append({"x": shard, "wblob": wblob})
    return in_maps


def kernel(input, W1, b1, W2, b2):
    global _CACHED_NC
    if _CACHED_NC is None:
        _CACHED_NC = build_bass()
    nc = _CACHED_NC

    in_maps = make_in_maps(input, W1, b1, W2, b2)
    res = run_bass_kernel_spmd(nc, in_maps, list(range(NCORES)))

    out = np.empty((B, C), dtype=np.float32)
    for i in range(NCORES):
        o = res.results[i]["out"]  # [4, 128*BPC], out[b, 128r+q] = o[r, 128b+q]
        out[BPC * i : BPC * (i + 1)] = (
            o.reshape(4, BPC, 128).transpose(1, 0, 2).reshape(BPC, C)
        )
    return out
